# revision 1
# baseline (speedup 1.0000x reference)
"""CrossLayerAttention Trainium2 Bass kernel (v2).

Math (folded form of the reference, scales s_l, temperature t):
  M  = Wq^T @ Wk / (sqrt(D)*|t|)       [D,D]
  qm = x_cur @ M                       [N,D]
  sc[n,l] = sum_d qm[n,d] * x_l[n,d]
  e  = exp(sc)                         (softmax max-sub skipped: |sc| < ~0.5)
  out[n,d] = sum_l s_l^2 e[n,l] x_l[n,d] / sum_l e[n,l] (s_l + 1e-6)

Sharding: data-parallel over tokens (N = B*T*H = 131072) across 8 cores.
Per-core layout: chunks of 1024 tokens; 128 partitions x 8 token-slots (J);
each token's 64 features contiguous in the free dim.

Engine split (per chunk, balanced against the 360GB/s DMA budget):
  DMA : x_all loaded as fp16 via SWDGE cast (2 chunks per dma_start to
        amortize the SWDGE fixed cost on Pool), x_cur f32 HWDGE, out f32.
  PE  : x_cur transposes + qm matmuls (fp16); output accumulation as 12
        identity-diag matmuls (W_l = diag(s_l^2)) into PSUM.
  DVE : fused mul+prefix-scan (custom op) for the first K layers of the
        scores pass; fp16 2x-mode products for the remaining layers and for
        the whole output pass; boundary diffs; denominator reduce/recip;
        final PSUM*(1/denom) -> SBUF.
  Pool: native tensor_tensor_scan over the remaining scores layers + diffs;
        e*(s+eps) multiply for the denominator.
  ACT : exp() expanded along d via a broadcast read (enables the 2x-mode
        output products); PSUM->SBUF copies in the qm phase.
"""

import os
import sys

import numpy as np

sys.path.insert(0, "/opt/trn_rl_repo")

L, B, T, H, D = 12, 4, 2048, 16, 64
N = B * T * H          # 131072 tokens
NCORES = 8
NTOK = N // NCORES     # 16384 tokens per core
P = 128                # partitions
CHUNK = 1024           # tokens per chunk
J = CHUNK // P         # 8 token-slots per partition
FD = J * D             # 512 free elems per layer
NCHUNK = NTOK // CHUNK # 16
LFD = L * FD           # 6144

# output products split: the last PPOOL layers are formed on Pool, the rest
# on DVE (fp16 2x mode).
PPOOL = int(os.environ.get("PPOOL", "7"))
POOL_STT = bool(int(os.environ.get("POOL_STT", "0")))
POOL_DIV = bool(int(os.environ.get("POOL_DIV", "0")))
LOAD_CHUNKS = int(os.environ.get("LOAD_CHUNKS", "2"))  # chunks per x_all dma

LAST_EXEC_NS = None
_CACHE = {}
OP_LOG = {}


def _lbl(label, inst):
    try:
        OP_LOG[inst.ins.name] = label
    except Exception:
        pass
    return inst


def _ap(base, offset_elems, dims, bass_mod):
    """AP over base tile's tensor: free dims list [(stride, count), ...]."""
    part = list(base.ap[0])
    return bass_mod.AP(
        tensor=base.tensor,
        offset=base.offset + offset_elems,
        ap=[part] + [list(d) for d in dims],
    )


def _register_mul_scan():
    from concourse import dve_ops
    from concourse.dve_spec import Spec, Src0, Src1, AluOp, scan, lower, _has_src1
    from concourse.dve_uop import DveOpSpec

    for op in dve_ops.OPS:
        if op.name == "MUL_SCAN_ANT":
            return op
    spec = Spec(
        body=scan(AluOp.ADD, Src0 * Src1),
        reference=lambda in0, in1, s0, s1, imm2: np.cumsum(
            (in0.astype(np.float32) * in1.astype(np.float32)).reshape(
                in0.shape[0], -1
            ),
            axis=-1,
        ).reshape(in0.shape),
    )
    name = "MUL_SCAN_ANT"
    row = 1 + len(dve_ops.OPS)
    dve_ops._SUB_OPCODE_FOR_NAME[name] = row
    shas = {}
    for ver in ("v3", "v4"):
        uops = lower(spec, ver=ver)
        s = DveOpSpec(name=name, opcode=row, uops=uops, rd1_en=_has_src1(spec))
        shas[ver] = s.sha(ver)
    op = dve_ops.DveOp(name, spec, subdim=False, uops_sha=shas)
    dve_ops.OPS.append(op)
    dve_ops.CUSTOM_DVE_SPECS[name] = spec
    return op


def _build():
    import concourse.bass as bass
    import concourse.bacc as bacc
    import concourse.tile as tile
    from concourse import mybir

    f32 = mybir.dt.float32
    f16 = mybir.dt.float16
    AF = mybir.ActivationFunctionType
    OP = mybir.AluOpType
    AX = mybir.AxisListType

    mul_scan = _register_mul_scan()
    ppool = PPOOL
    assert 0 <= ppool < L
    LDVE_PO = L - ppool       # po layers on DVE

    nc = bacc.Bacc("TRN2", target_bir_lowering=False)

    x_cur_d = nc.dram_tensor("x_cur", [NTOK, D], f32, kind="ExternalInput")
    x_all_d = nc.dram_tensor("x_all", [L, NTOK, D], f32, kind="ExternalInput")
    # host passes blockdiag(Wq, Wq) / blockdiag(Wk, Wk) [128,128]
    wq_d = nc.dram_tensor("wq", [P, P], f32, kind="ExternalInput")
    wk_d = nc.dram_tensor("wk", [P, P], f32, kind="ExternalInput")
    scales_d = nc.dram_tensor("scales", [1, L], f32, kind="ExternalInput")
    temp_d = nc.dram_tensor("temp", [1, 1], f32, kind="ExternalInput")
    ident_d = nc.dram_tensor("ident", [P, P], f32, kind="ExternalInput")
    out_d = nc.dram_tensor("out", [NTOK, D], f32, kind="ExternalOutput")

    # DRAM views: superchunk s = LOAD_CHUNKS*1024 tokens; token t maps to
    # partition p, slot jj (t = s*LC*1024 + p*LC*8 + jj). Chunk cc within a
    # superchunk takes jj in [cc*8, cc*8+8).
    NSC = NCHUNK // LOAD_CHUNKS
    JJ = J * LOAD_CHUNKS
    x_cur_v = x_cur_d[:].rearrange("(s p jj) d -> s p (jj d)", s=NSC, p=P, jj=JJ)
    x_all_v = x_all_d[:].rearrange(
        "l (s p jj) d -> s p l (jj d)", s=NSC, p=P, jj=JJ
    )
    out_v = out_d[:].rearrange("(s p jj) d -> s p (jj d)", s=NSC, p=P, jj=JJ)

    with tile.TileContext(nc) as tc:
        with (
            tc.tile_pool(name="singles", bufs=1) as singles,
            tc.tile_pool(name="xall", bufs=int(os.environ.get("XALL_BUFS", "3"))) as xall_pool,
            tc.tile_pool(name="io", bufs=int(os.environ.get("IO_BUFS", "4"))) as io_pool,
            tc.tile_pool(name="qm", bufs=int(os.environ.get("QM_BUFS", "2"))) as qm_pool,
            tc.tile_pool(name="prodo", bufs=int(os.environ.get("PRODO_BUFS", "2"))) as prodo_pool,
            tc.tile_pool(name="eexp", bufs=int(os.environ.get("EEXP_BUFS", "3"))) as eexp_pool,
            tc.tile_pool(name="sm", bufs=int(os.environ.get("SM_BUFS", "4"))) as sm_pool,
            tc.tile_pool(name="ot", bufs=int(os.environ.get("OT_BUFS", "3"))) as ot_pool,
            tc.tile_pool(name="psum", bufs=2, space="PSUM") as psum_pool,
            tc.tile_pool(name="psone", bufs=1, space="PSUM") as psone_pool,
            tc.tile_pool(name="psacc", bufs=int(os.environ.get("PSACC_BUFS", "3")), space="PSUM") as psacc_pool,
        ):
            # ---- one-time preamble -------------------------------------
            ident = singles.tile([P, P], f32)
            nc.sync.dma_start(out=ident[:], in_=ident_d[:])

            wq_sb = singles.tile([P, P], f32)
            wk_sb = singles.tile([P, P], f32)
            nc.sync.dma_start(out=wq_sb[:], in_=wq_d[:])
            nc.sync.dma_start(out=wk_sb[:], in_=wk_d[:])

            scales_sb = singles.tile([P, L], f32)
            nc.sync.dma_start(
                out=scales_sb[:],
                in_=bass.AP(tensor=scales_d, offset=0, ap=[[0, P], [1, L]]),
            )

            # inv_scale = 1/(sqrt(D)*|temp|) on all partitions
            temp_sb = singles.tile([P, 1], f32)
            nc.sync.dma_start(
                out=temp_sb[:],
                in_=bass.AP(tensor=temp_d, offset=0, ap=[[0, P], [1, 1]]),
            )
            t8 = singles.tile([P, 1], f32)
            nc.scalar.activation(t8[:], temp_sb[:], AF.Abs, scale=float(np.sqrt(D)))
            inv_bc = singles.tile([P, 1], f32)
            nc.vector.reciprocal(inv_bc[:], t8[:])

            # blockdiag(M, M) = blockdiag(Wq,Wq)^T @ blockdiag(Wk,Wk); the
            # inv_scale multiply + fp16 cast happen in the PSUM->SBUF copy.
            m2_ps = psone_pool.tile([P, P], f32, tag="m2ps")
            nc.tensor.matmul(m2_ps[:], wq_sb[:], wk_sb[:])
            m2h = singles.tile([P, P], f16)
            nc.scalar.activation(m2h[:], m2_ps[:], AF.Copy, scale=inv_bc[:])

            # identity fp16 (for transposes), per-layer diag(s_l^2) weights
            identh = singles.tile([P, P], f16)
            nc.scalar.copy(identh[:], ident[:])
            scales_pe = singles.tile([P, L], f32)   # s_l + 1e-6
            nc.vector.tensor_scalar(
                out=scales_pe[:], in0=scales_sb[:], scalar1=1e-6, scalar2=None,
                op0=OP.add,
            )
            scales_sq = singles.tile([P, L], f32)
            nc.vector.tensor_tensor(
                out=scales_sq[:], in0=scales_sb[:], in1=scales_sb[:], op=OP.mult
            )
            # gate: Pool's first op reads temp_sb so the x_all SWDGE gen
            # (and hence its DMA-engine acquisition) happens only after the
            # small preamble DMAs have cleared the queue.
            pool_gate = singles.tile([P, 1], f32)
            nc.gpsimd.tensor_tensor(out=pool_gate[:], in0=temp_sb[:],
                                    in1=temp_sb[:], op=OP.mult)

            wl = singles.tile([P, L, P], f16)

            def build_wl():
                for l in range(L):
                    nc.scalar.activation(
                        wl[:, l, :], ident[:], AF.Copy,
                        scale=scales_sq[:, l:l + 1],
                    )

            # persistent prefix tiles (seed col zeroed once; scans write
            # offsets >= 1 only)
            n_dp = int(os.environ.get("DVE_PREFIX_TILES", "2"))
            dve_prefix = []
            for i in range(n_dp):
                t = singles.tile([P, 1 + LFD], f16, tag=f"dvp_{i}")
                nc.vector.memset(t[:, 0:1], 0.0)
                dve_prefix.append(t)

            # ---- main loop: 3-stage software pipeline -------------------
            # iteration i:  S1 qm-phase for chunk i; S2 scores for i-1;
            #               S3 softmax+output for i-2.
            # x_all groups are prefetched PF groups ahead.
            PF = int(os.environ.get("PF_GROUPS", "1"))
            NG = NCHUNK // LOAD_CHUNKS
            xt2_tiles = {}
            xc2_tiles = {}

            def load_group(g):
                if g < 0 or g >= NG:
                    return
                xc2 = io_pool.tile([P, LOAD_CHUNKS * FD], f32, tag="xc")
                _lbl(f"load.xcur[{g}]", nc.sync.dma_start(out=xc2[:], in_=x_cur_v[g]))
                xt2 = xall_pool.tile([P, L, LOAD_CHUNKS * FD], f16, tag="xt2")
                if g == 0 and LOAD_CHUNKS > 1:
                    # split the first group per chunk so chunk 0's scores can
                    # start after half the transfer
                    for cc in range(LOAD_CHUNKS):
                        _lbl(f"load.xall[{g}.{cc}]", nc.gpsimd.dma_start(
                            out=xt2[:, :, cc * FD:(cc + 1) * FD],
                            in_=x_all_v[g][:, :, cc * FD:(cc + 1) * FD],
                        ))
                else:
                    _lbl(f"load.xall[{g}]", nc.gpsimd.dma_start(
                        out=xt2[:], in_=x_all_v[g]))
                xt2_tiles[g] = xt2
                xc2_tiles[g] = xc2

            def chunk_views(c):
                g, cc = divmod(c, LOAD_CHUNKS)
                xt = xt2_tiles[g][:, :, cc * FD:(cc + 1) * FD]
                xc = xc2_tiles[g][:, cc * FD:(cc + 1) * FD]
                return xt, xc

            qm_tiles = {}
            sc_tiles = {}
            r_tiles = {}

            def s1_qm(c):
                if c < 0 or c >= NCHUNK:
                    return
                _, xc = chunk_views(c)
                tp_ps = psum_pool.tile([P, FD], f32, tag="tp_ps")
                for h in range(J // 2):
                    nc.tensor.transpose(
                        tp_ps[:, h * P:(h + 1) * P],
                        xc[:, h * P:(h + 1) * P], ident[:],
                    )
                xts = qm_pool.tile([P, FD], f16, tag="xts")
                _lbl(f"s1.xts[{c}]", nc.scalar.copy(xts[:], tp_ps[:]))
                qm_ps = psum_pool.tile([P, FD], f32, tag="qm_ps")
                for h in range(J // 2):
                    nc.tensor.matmul(
                        qm_ps[:, h * P:(h + 1) * P],
                        xts[:, h * P:(h + 1) * P], m2h[:],
                    )
                qm = qm_pool.tile([P, FD], f16, tag="qmt")
                _lbl(f"s1.qmcopy[{c}]", nc.scalar.copy(qm[:], qm_ps[:]))
                qm_tiles[c] = qm

            eexp_tiles = {}

            def s2_scores(c):
                if c < 0 or c >= NCHUNK:
                    return
                xt, _ = chunk_views(c)
                qm = qm_tiles.pop(c)
                sc = sm_pool.tile([P, L, J], f32, tag="sc")
                dvp = dve_prefix[c % n_dp]
                _lbl(f"s2.dvescan[{c}]", nc.vector._custom_dve(
                    mul_scan,
                    out=_ap(dvp[:], 1, [[1, LFD]], bass),
                    in0=xt[:],
                    in1=_ap(qm[:], 0, [[0, L], [1, FD]], bass),
                ))
                deng = nc.gpsimd if bool(int(os.environ.get("DIFFS_POOL", "0"))) else nc.vector
                _lbl(f"s2.ddiffs[{c}]", deng.tensor_sub(
                    sc[:].rearrange("p l j -> p (l j)"),
                    _ap(dvp[:], D, [[D, L * J]], bass),
                    _ap(dvp[:], 0, [[D, L * J]], bass),
                ))
                sc_tiles[c] = sc

            def s3a_softmax(c):
                if c < 0 or c >= NCHUNK:
                    return
                sc = sc_tiles.pop(c)
                # e = exp(sc) (small), denominator, r = 1/den, then fold r
                # into the scores: sc2 = sc + ln(r) so that exp(sc2) = e*r.
                es = sm_pool.tile([P, L * J], f16, tag="es")
                _lbl(f"s3a.esmall[{c}]", nc.scalar.activation(
                    es[:], sc[:].rearrange("p l j -> p (l j)"), AF.Exp
                ))
                t96 = sm_pool.tile([P, L * J], f32, tag="t96")
                _lbl(f"s3a.t96[{c}]", nc.gpsimd.tensor_tensor(
                    out=t96[:],
                    in0=es[:],
                    in1=_ap(scales_pe[:], 0, [[1, L], [0, J]], bass),
                    op=OP.mult,
                ))
                den = sm_pool.tile([P, J], f32, tag="den")
                _lbl(f"s3a.den[{c}]", nc.vector.tensor_reduce(
                    op=OP.add, out=den[:],
                    in_=_ap(t96[:], 0, [[1, J], [J, L]], bass), axis=AX.X,
                ))
                if bool(int(os.environ.get("DVE_DIV", "0"))):
                    r = den
                else:
                    r = sm_pool.tile([P, J], f32, tag="r")
                    _lbl(f"s3a.recip[{c}]", nc.vector.reciprocal(r[:], den[:]))
                # expand exp(sc) along d for the DVE po layers only; the
                # Pool po layers read es broadcast (no 2x mode to protect).
                eexp = eexp_pool.tile([P, LDVE_PO * J, D], f16, tag="eexp")
                scb = _ap(sc[:], 0, [[J, LDVE_PO], [1, J], [0, D]], bass)
                _lbl(f"s3a.expand[{c}]", nc.scalar.activation(
                    eexp[:].rearrange("p lj d -> p (lj d)"), scb, AF.Exp
                ))
                if POOL_DIV and ppool:
                    esn = sm_pool.tile([P, ppool * J], f16, tag="esn")
                    _lbl(f"s3a.esneg[{c}]", nc.scalar.activation(
                        esn[:],
                        sc[:, LDVE_PO:L, :].rearrange("p l j -> p (l j)"),
                        AF.Exp, scale=-1.0,
                    ))
                else:
                    esn = es
                eexp_tiles[c] = (eexp, esn)
                r_tiles[c] = r

            po_tiles = {}

            def s3b_po(c):
                if c < 0 or c >= NCHUNK:
                    return
                xt, _ = chunk_views(c)
                eexp, e2 = eexp_tiles.pop(c)
                po = prodo_pool.tile([P, L, FD], f16, tag="po")
                if ppool:
                    nsp = int(os.environ.get("PPO_SPLIT", "1"))
                    bnds = [LDVE_PO + (ppool * k) // nsp for k in range(nsp + 1)]
                    for k in range(nsp):
                        l0, l1 = bnds[k], bnds[k + 1]
                        if l0 == l1:
                            continue
                        _lbl(f"s3b.ppo[{c}.{k}]", nc.gpsimd.tensor_tensor(
                            out=po[:, l0:l1, :],
                            in0=xt[:, l0:l1, :],
                            in1=_ap(e2[:], l0 * J,
                                    [[J, l1 - l0], [1, J], [0, D]], bass),
                            op=OP.mult,
                        ))
                _lbl(f"s3b.prodo[{c}]", nc.vector.tensor_tensor(
                    out=po[:, 0:LDVE_PO, :],
                    in0=xt[:, 0:LDVE_PO, :],
                    in1=eexp[:].rearrange("p (l j) d -> p l (j d)", l=LDVE_PO),
                    op=OP.mult,
                ))
                po_tiles[c] = po

            acc_tiles = {}

            def s3b_fin(c):
                if c < 0 or c >= NCHUNK:
                    return
                po = po_tiles.pop(c)
                acc_ps = psacc_pool.tile([P, FD], f32, tag="acc")
                for l in range(L):
                    _lbl(f"s3b.mm[{c}.{l}]", nc.tensor.matmul(
                        acc_ps[:], wl[:, l, :], po[:, l, :],
                        start=(l == 0), stop=(l == L - 1),
                    ))
                acc_tiles[c] = acc_ps

            FDIV_ACT = int(os.environ.get("FDIV_ACT", "0"))

            def s3c_out(c):
                if c < 0 or c >= NCHUNK:
                    return
                g, cc = divmod(c, LOAD_CHUNKS)
                acc_ps = acc_tiles.pop(c)
                r = r_tiles.pop(c)
                ot = ot_pool.tile([P, FD], f32, tag="ot")
                if bool(int(os.environ.get("DVE_DIV", "0"))):
                    _lbl(f"s3c.fdiv[{c}]", nc.vector.tensor_tensor(
                        out=ot[:], in0=acc_ps[:],
                        in1=_ap(r[:], 0, [[1, J], [0, D]], bass),
                        op=OP.divide,
                    ))
                    _lbl(f"s3c.store[{c}]", nc.sync.dma_start(
                        out=out_v[g][:, cc * FD:(cc + 1) * FD], in_=ot[:]
                    ))
                    return
                npl = int(os.environ.get("FDIV_POOL", "0"))
                if npl:
                    _lbl(f"s3c.pfdiv[{c}]", nc.gpsimd.tensor_tensor(
                        out=ot[:, (J - npl) * D:],
                        in0=acc_ps[:, (J - npl) * D:],
                        in1=_ap(r[:], J - npl, [[1, npl], [0, D]], bass),
                        op=OP.mult,
                    ))
                    _lbl(f"s3c.fdiv[{c}]", nc.vector.tensor_tensor(
                        out=ot[:, 0:(J - npl) * D], in0=acc_ps[:, 0:(J - npl) * D],
                        in1=_ap(r[:], 0, [[1, J - npl], [0, D]], bass),
                        op=OP.mult,
                    ))
                    _lbl(f"s3c.store[{c}]", nc.sync.dma_start(
                        out=out_v[g][:, cc * FD:(cc + 1) * FD], in_=ot[:]
                    ))
                    return
                na = FDIV_ACT
                for j in range(na):
                    _lbl(f"s3c.afdiv[{c}.{j}]", nc.scalar.activation(
                        ot[:, j * D:(j + 1) * D], acc_ps[:, j * D:(j + 1) * D],
                        AF.Copy, scale=r[:, j:j + 1],
                    ))
                if na < J:
                    _lbl(f"s3c.fdiv[{c}]", nc.vector.tensor_tensor(
                        out=ot[:, na * D:], in0=acc_ps[:, na * D:],
                        in1=_ap(r[:], na, [[1, J - na], [0, D]], bass),
                        op=OP.mult,
                    ))
                _lbl(f"s3c.store[{c}]", nc.sync.dma_start(
                    out=out_v[g][:, cc * FD:(cc + 1) * FD], in_=ot[:]
                ))

            for g in range(PF + 1):
                load_group(g)
            for i in range(NCHUNK + 4):
                if i < NCHUNK and i % LOAD_CHUNKS == 0:
                    load_group(i // LOAD_CHUNKS + PF + 1)
                s3b_po(i - 3)
                s3c_out(i - 4)
                s1_qm(i)
                if i == 1:
                    build_wl()
                s2_scores(i - 1)
                s3a_softmax(i - 2)
                s3b_fin(i - 3)

    nc.compile()
    return nc


def _get_nc():
    if "nc" not in _CACHE:
        _CACHE["nc"] = _build()
    return _CACHE["nc"]


def kernel(current_layer, all_layers, Wq, Wk, scales, temperature, current_layer_idx=0):
    nc = _get_nc()
    from concourse.bass_utils import run_bass_kernel_spmd

    x_cur = np.ascontiguousarray(np.asarray(current_layer, np.float32).reshape(N, D))
    x_all = np.ascontiguousarray(np.asarray(all_layers, np.float32).reshape(L, N, D))
    wq1 = np.asarray(Wq, np.float32)
    wk1 = np.asarray(Wk, np.float32)
    wq = np.zeros((P, P), np.float32)
    wk = np.zeros((P, P), np.float32)
    wq[:D, :D] = wq1; wq[D:, D:] = wq1
    wk[:D, :D] = wk1; wk[D:, D:] = wk1
    sc = np.ascontiguousarray(np.asarray(scales, np.float32).reshape(1, L))
    tp = np.ascontiguousarray(np.asarray(temperature, np.float32).reshape(1, 1))
    ident = np.eye(P, dtype=np.float32)

    in_maps = []
    for c in range(NCORES):
        sl = slice(c * NTOK, (c + 1) * NTOK)
        in_maps.append({
            "x_cur": x_cur[sl],
            "x_all": np.ascontiguousarray(x_all[:, sl]),
            "wq": wq, "wk": wk, "scales": sc, "temp": tp, "ident": ident,
        })

    trace = bool(int(os.environ.get("KERNEL_TRACE", "0")))
    res = run_bass_kernel_spmd(nc, in_maps, core_ids=list(range(NCORES)), trace=trace)

    global LAST_EXEC_NS
    LAST_EXEC_NS = res.exec_time_ns

    out = np.empty((N, D), np.float32)
    for c in range(NCORES):
        out[c * NTOK:(c + 1) * NTOK] = res.results[c]["out"]
    return out.reshape(B, T, H, D)

